# revision 1
# baseline (speedup 1.0000x reference)
"""Trainium2 Bass kernel for nn_DistributionLoss_6940667150680 (segment_reduce).

Math: with per-class sums S_c = sum_{i: Y_i=c} w_i and counts n_c,
    L2 = sum_i ||w_i - S_{Y_i}/n_{Y_i}||^2 = sum_i ||w_i||^2 - sum_c ||S_c||^2/n_c
so a single streaming pass over w1 suffices.

Sharding strategy (segment-key sharding): the host routes rows by class --
rows are stably sorted by label and each class is padded with zero rows to a
multiple of 128 so that every 128-row tile belongs to exactly one class.  The
padded tile stream is split evenly across the 8 cores.  Each core then only
needs per-TILE column sums (S_c = sum of its tiles' sums, reduced on host),
which turns the segment reduction into a dense streaming reduce:

  - PE: per tile one matmul with a constant selector mask e_i [128, 32]
    (column i all-ones) as the stationary operand: psum accumulates
    e_i^T @ w_tile, i.e. the tile's column sum lands in psum row i and zeros
    elsewhere (M=32, N=128, fp16 in / f32 psum; 32-matmul accumulation group
    per chunk).  No per-tile weight loads of w, no one-hot build on DVE.
  - ACT: Square activation with accum_out on a slice of each chunk.
  - DVE: fused tensor_tensor_reduce (w*w, sum) on the rest, plus tiny
    psum->SBUF evacuations of the per-tile sums.
  - Host: fp16 cast + class-sorted layout (input prep), per-class reduction
    of tile sums, counts via bincount, final scalar in float64.
"""

import ml_dtypes
import numpy as np
from contextlib import ExitStack

import concourse.bass as bass
import concourse.tile as tile
from concourse import mybir
from concourse.bass_utils import run_bass_kernel_spmd

N_CORES = 8
D = 128          # feature dim
P = 128          # partitions / rows per tile
CHUNK = 32       # tiles per DMA chunk
RING = 6         # w-ring depth in chunks
NPS = 4          # psum tiles (round-robin per chunk)
USE_FP8 = True   # stream w as fp8 e4m3 (halves HBM traffic; rel err ~7e-4)
ACT_COLS = 2112 if USE_FP8 else 2560  # ACT's share of each chunk's columns
NP_DT = ml_dtypes.float8_e4m3 if USE_FP8 else np.float16


def build_program(T: int, act_cols: int = ACT_COLS):
    """Per-core program processing T tiles (T % CHUNK == 0)."""
    f32, f16 = mybir.dt.float32, mybir.dt.float16
    fdt = mybir.dt.float8e4 if USE_FP8 else f16
    assert T % CHUNK == 0
    nch = T // CHUNK
    CF = CHUNK * D           # columns per chunk
    fdA = min(act_cols, CF)
    fdV = CF - fdA

    nc = bass.Bass()
    w_in = nc.dram_tensor("w", [P, T * D], fdt, kind="ExternalInput")
    masks_in = nc.dram_tensor("masks", [P, CHUNK * CHUNK], fdt, kind="ExternalInput")
    ts_out = nc.dram_tensor("ts_out", [CHUNK, nch * D], f32, kind="ExternalOutput")
    sqa_out = nc.dram_tensor("sqa_out", [P, nch], f32, kind="ExternalOutput")
    sqv_out = nc.dram_tensor("sqv_out", [P, max(nch, 1)], f32, kind="ExternalOutput")

    def dep(frm, to, why):
        tile.add_dep_helper(
            getattr(frm, "ins", frm), getattr(to, "ins", to), reason=why
        )

    def demote(inst, dep_insts):
        """Move provably-redundant sync deps to nosync (ordering only):
        same-engine WAW/WAR (in-order engines) and deps transitively covered
        by another emitted wait (ISA structs hold one sync wait each)."""
        inst = getattr(inst, "ins", inst)
        drop = set()
        for d in dep_insts:
            if d is None:
                continue
            drop.add(getattr(d, "ins", d).name)
        syncs = inst.take_sync_dependencies()
        nosyncs = inst.take_nosync_dependencies()
        for name in drop & set(syncs):
            syncs.discard(name)
            nosyncs.add(name)
        inst.set_sync_dependencies(syncs)
        inst.set_nosync_dependencies(nosyncs)

    # Pin each engine queue to emission order with demoted (nosync) chain
    # edges: the tile scheduler may otherwise reorder within a queue, which
    # breaks every "covered transitively via in-order engine" argument below.
    last_on = {}

    def chain(inst, engine):
        prev = last_on.get(engine)
        if prev is not None:
            dep(inst, prev, "queue order")
            demote(inst, [prev])
        last_on[engine] = inst
        return inst

    with tile.TileContext(nc) as tc, ExitStack() as ctx:
        const = ctx.enter_context(tc.tile_pool(name="const", bufs=1))
        psum = ctx.enter_context(tc.tile_pool(name="psum", bufs=1, space="PSUM"))

        masks_sb = const.tile([P, CHUNK * CHUNK], fdt, name="masks_sb")
        dma_masks = nc.sync.dma_start(out=masks_sb, in_=masks_in[:, :])
        w_ring = const.tile([P, RING, CF], fdt, name="w_ring")
        sqa_cols = const.tile([P, nch], f32, name="sqa_cols")
        sqv_cols = const.tile([P, max(nch, 1)], f32, name="sqv_cols")
        scrA = const.tile([P, 2, fdA], f16, name="scrA")  # f16 out: accum runs fp32 internally
        scrV = const.tile([P, 2, max(fdV, 2)], f16, name="scrV")
        out_sb = const.tile([CHUNK, nch * D], f32, name="out_sb")
        touch = const.tile([1, 4], f32, name="touch")

        pst = [psum.tile([CHUNK, D], f32, name=f"pst{k}") for k in range(NPS)]

        # DVE touch of the mask const so the first matmul needs only one wait.
        t_masks = chain(nc.vector.tensor_copy(touch[0:1, 0:1], masks_sb[0:1, 0:1]),
                        "dve")
        touch_writers = [t_masks]

        copies = {}
        mm_last = {}
        acts = {}
        ttrs = {}
        dmas = {}
        readers = {}  # chunk -> every instruction that reads its ring slot

        for c in range(nch):
            j = c % RING
            slot = w_ring[:, j, :]
            # Ring-slot WAR: carry waits on SP nops (the chain edges keep the
            # DMA behind them in the SP queue, so their hardware waits also
            # protect it).
            if c >= RING:
                n1 = chain(nc.sync.nop(nofuse=True, hint=f"war{c}a"), "sp")
                dep(n1, acts[c - RING], "act reader done")
                if (c - RING) in ttrs:
                    n1b = chain(nc.sync.nop(nofuse=True, hint=f"war{c}c"), "sp")
                    dep(n1b, ttrs[c - RING], "ttr reader done")
                n2 = chain(nc.sync.nop(nofuse=True, hint=f"war{c}b"), "sp")
                dep(n2, mm_last[c - RING], "pe reader done")
            dma = chain(
                nc.sync.dma_start(out=slot, in_=w_in[:, c * CF : (c + 1) * CF]),
                "sp",
            )
            # WAR waits live on the nops just above; DMA-vs-DMA WAW is ordered
            # by the HWDGE ring (FIFO per issuing engine).  WAR deps can be
            # re-attached against ANY prior occupant's readers, so demote all.
            demote(dma, list(dmas.values())
                   + [r for k in range(c) for r in readers[k]])
            dmas[c] = dma
            readers[c] = []

            # DVE touch: carries this chunk's DMA wait; the chain edges pin it
            # after copy_{c-NPS} on the in-order DVE queue, so a wait on it
            # also covers the psum-tile WAR for this chunk's matmuls.
            tch = chain(nc.vector.tensor_copy(touch[0:1, 1:2], slot[0:1, 0:1]),
                        "dve")
            demote(tch, touch_writers)  # same-engine WAW on the touch tile
            touch_writers.append(tch)
            readers[c].append(tch)

            # PE: one matmul per tile; e_i^T @ w_tile accumulates the tile's
            # column sum into psum row i (zeros elsewhere).
            pt = pst[c % NPS]
            for i in range(CHUNK):
                mm = chain(
                    nc.tensor.matmul(
                        pt,
                        lhsT=masks_sb[:, i * CHUNK : (i + 1) * CHUNK],
                        rhs=slot[:, i * D : (i + 1) * D],
                        start=(i == 0),
                        stop=(i == CHUNK - 1),
                    ),
                    "pe",
                )
                if i == 0:
                    dep(mm, tch, "chunk + psum ready (transitive)")
                # i == 0: dma/psum-WAR covered via the touch; i > 0: covered
                # transitively (PE chain keeps them behind mm[0]).
                demote(mm, [dma, dma_masks, t_masks, copies.get(c - NPS),
                            mm_last.get(c - NPS)])
                readers[c].append(mm)
            mm_last[c] = mm

            # ACT: squares of the first fdA columns, accumulated per chunk.
            act = chain(
                nc.scalar.activation(
                    scrA[:, c % 2, :],
                    slot[:, 0:fdA],
                    mybir.ActivationFunctionType.Square,
                    accum_out=sqa_cols[:, c : c + 1],
                ),
                "act",
            )
            demote(act, list(acts.values()))  # same-engine WAW on scrA
            acts[c] = act
            readers[c].append(act)

            # DVE: fused square+reduce of the remaining columns.  Carries its
            # own DMA wait (it may not sit right behind the touch).
            if fdV > 0:
                ttr = chain(
                    nc.vector.scalar_tensor_tensor(
                        out=scrV[:, c % 2, :],
                        in0=slot[:, fdA:CF],
                        scalar=1.0,
                        in1=slot[:, fdA:CF],
                        op0=mybir.AluOpType.mult,
                        op1=mybir.AluOpType.mult,
                        accum_out=sqv_cols[:, c : c + 1],
                    ),
                    "dve",
                )
                demote(ttr, list(ttrs.values()))  # scrV WAW same-engine
                ttrs[c] = ttr
                readers[c].append(ttr)

            # DVE: evacuate this chunk's tile sums psum -> SBUF.
            cp = chain(nc.vector.tensor_copy(out_sb[:, c * D : (c + 1) * D], pt),
                       "dve")
            demote(cp, [mm_last[k] for k in range(c)] + list(copies.values()))
            copies[c] = cp

        # Outputs: each DMA waits on the last producer via an SP nop (the
        # producing engines are in-order, so last implies all).
        outs = []
        for name, buf, last in (
            ("ts", ts_out, copies[nch - 1]),
            ("sqa", sqa_out, acts[nch - 1]),
            ("sqv", sqv_out, ttrs.get(nch - 1)),
        ):
            if last is None:
                continue
            spn = chain(nc.sync.nop(nofuse=True, hint=f"out_{name}"), "sp")
            dep(spn, last, f"{name} ready")
            src = {"ts": out_sb, "sqa": sqa_cols, "sqv": sqv_cols}[name]
            od = chain(nc.sync.dma_start(out=buf[:, :], in_=src), "sp")
            dep(od, spn, "after producer nop")
            demote(od, [spn, last] + list(dmas.values()) + outs
                   + list(copies.values()) + list(acts.values())
                   + list(ttrs.values()))
            outs.append(od)

        # Tail sync: cover every proc with single-wait SP nops.
        tails = [mm_last[nch - 1], acts[nch - 1], copies[nch - 1]] + outs
        if (nch - 1) in ttrs:
            tails.append(ttrs[nch - 1])
        for t in tails:
            nop = chain(nc.sync.nop(nofuse=True, hint="tailcover"), "sp")
            dep(nop, t, "tail")

    # The kernel-tail drain waits on every proc; its NOP struct cannot hold
    # that many sync waits and the SP-queue nops above already cover them.
    for blk in nc.m.functions[0].blocks:
        for inst in blk.instructions:
            if not isinstance(inst, mybir.InstDrain):
                continue
            si = inst.sync_info
            if si is None or len(si.on_wait) <= 2:
                continue
            inst.sync_info = mybir.SyncInfo(on_wait=[], on_update=list(si.on_update))

    return nc


def prepare_inputs(w1: np.ndarray, Y: np.ndarray, num_classes: int):
    """Class-sorted, per-class tile-padded, per-core partition-major fp16."""
    n = w1.shape[0]
    counts = np.bincount(Y, minlength=num_classes).astype(np.int64)
    tpc_class = (counts + P - 1) // P          # tiles per class
    pad_start = np.zeros(num_classes + 1, dtype=np.int64)
    np.cumsum(tpc_class, out=pad_start[1:])
    tt = int(pad_start[-1])                    # total real tiles
    t_core = -(-tt // N_CORES)                 # ceil
    t_core = -(-t_core // CHUNK) * CHUNK       # round up to chunk
    t_total = t_core * N_CORES

    order = np.argsort(Y, kind="stable")
    y_sorted = Y[order]
    class_start = np.zeros(num_classes, dtype=np.int64)
    class_start[1:] = np.cumsum(counts)[:-1]
    rank = np.arange(n, dtype=np.int64) - class_start[y_sorted]
    dest = pad_start[y_sorted] * P + rank

    w16 = np.zeros((t_total * P, D), dtype=NP_DT)
    w16[dest] = w1[order].astype(NP_DT)

    # selector masks: mask_i[r, m] = (m == i), laid out [P, i*CHUNK + m]
    masks = np.ascontiguousarray(
        np.broadcast_to(np.eye(CHUNK, dtype=NP_DT).reshape(1, CHUNK * CHUNK),
                        (P, CHUNK * CHUNK))
    )
    in_maps = []
    for k in range(N_CORES):
        blk = w16[k * t_core * P : (k + 1) * t_core * P]
        wk = np.ascontiguousarray(
            blk.reshape(t_core, P, D).transpose(1, 0, 2).reshape(P, t_core * D)
        )
        in_maps.append({"w": wk, "masks": masks})
    return in_maps, t_core, pad_start, counts


def combine(results, t_core, pad_start, counts, n_total):
    """Host-side: tile sums -> class sums -> final scalar, in float64."""
    nch = t_core // CHUNK
    tile_sums = np.concatenate(
        [
            r["ts_out"].astype(np.float64)
            .reshape(CHUNK, nch, D).transpose(1, 0, 2).reshape(t_core, D)
            for r in results
        ],
        axis=0,
    )  # [t_total, D]
    num_classes = len(counts)
    totsq = 0.0
    for r in results:
        totsq += float(r["sqa_out"].astype(np.float64).sum())
        totsq += float(r["sqv_out"].astype(np.float64).sum())
    # per-class sums: classes are tile-aligned runs of tile_sums
    corr = 0.0
    seg = np.add.reduceat(tile_sums[: pad_start[-1]], pad_start[:-1], axis=0) \
        if pad_start[-1] > 0 else np.zeros((num_classes, D))
    # reduceat quirk: empty segments (pad_start[c]==pad_start[c+1]) copy the
    # row at that index instead of 0 -- mask them out via counts.
    nz = counts > 0
    s = seg[nz]
    corr = float(((s * s).sum(axis=1) / counts[nz]).sum())
    return np.float32((totsq - corr) / n_total)


def run_sharded(w1: np.ndarray, Y: np.ndarray, num_classes: int, trace: bool = False):
    w1 = np.ascontiguousarray(np.asarray(w1, dtype=np.float32))
    Y = np.asarray(Y).astype(np.int64)
    in_maps, t_core, pad_start, counts = prepare_inputs(w1, Y, num_classes)
    nc = build_program(t_core)
    out = run_bass_kernel_spmd(nc, in_maps, list(range(N_CORES)), trace=trace)
    value = combine(out.results, t_core, pad_start, counts, w1.shape[0])
    return value, out


def kernel(w1, Y, num_classes=None):
    w1 = np.asarray(w1, dtype=np.float32)
    Y = np.asarray(Y)
    c = int(np.asarray(num_classes)) if num_classes is not None else 1000
    assert w1.ndim == 2 and w1.shape[1] == D
    value, _ = run_sharded(w1, Y, c, trace=False)
    return value

